# revision 18
# baseline (speedup 1.0000x reference)
"""Trainium2 Bass kernel for MatchingLayerL2:
   out = log_softmax(-sqrt(||x_i - y_j||^2) / std_j, axis=1)

x: [4096, 128] f32, y: [32768, 128] f32, std: [32768] f32 -> out [4096, 32768] f32.

Strategy: shard rows of x across 8 cores (512 rows each); y/std replicated.
Host prepares device inputs (layout/dtype prep only, O((N+M)D) work):
  yhatT = (y * r2[:,None]).T as bf16 [128, M]   (r2 = 1/std^2)
  xT    = (-2 x_c).T as bf16 [128, 512]
  corr rows (rank-2 term a_i*r2_j + bhat_j in hi/lo bf16 splits, K=5):
    cl = [a_hi; a_lo; a_hi; 1; 1]  [5, 512]
    cr = [r2_hi; r2_hi; r2_lo; bhat_hi; bhat_lo]  [5, M]
Device per core:
  q = xT.T @ yhatT + cl.T @ cr   (PSUM f32, = r2_j * dist2_ij)
  s = sqrt(q)  (fp16; split: 1/4 of chunks on ACT Sqrt, 3/4 via
               DVE copy PSUM->SBUF fp16 then GPSIMD tensor_tensor pow 0.5 —
               GPSIMD cannot read PSUM and sqrt/exp only exist on ACT/Pool)
  S_i = sum_j exp(-s)  (ACT Exp with accum, fp8 scratch out)
  out = -s - ln(S)     (DVE tensor_scalar in-place, fp16) -> DMA fp16
Engine balance target ~143us each for ACT (exp + 1/4 sqrt),
Pool (3/4 sqrt), DVE (copies + final); PE ~110us; DMA ~118us.
"""

import os
import sys

sys.path.insert(0, "/root/.axon_site/_ro/trn_rl_repo")

import numpy as np
import ml_dtypes
from contextlib import ExitStack

import concourse.bass as bass
from concourse import bacc
import concourse.tile as tile
from concourse.tile import add_dep_helper
from concourse import mybir
from concourse.bass_utils import run_bass_kernel_spmd

F32 = mybir.dt.float32
BF16 = mybir.dt.bfloat16
FP16 = mybir.dt.float16
FP8 = mybir.dt.float8e4
AF = mybir.ActivationFunctionType
ALU = mybir.AluOpType
AX = mybir.AxisListType

N_CORES = 8
D = 128
P = 128
CHUNK = 2048          # PSUM region columns (4 banks f32)
GROUP = 8192          # columns per exp instruction / s sub-tile
BF = ml_dtypes.bfloat16


def build_nc(rows, M):
    NB = rows // P            # 4 row blocks of 128
    NG = M // GROUP           # 4 groups per block
    NCP = GROUP // CHUNK      # 4 chunks per group

    nc = bacc.Bacc("TRN2", target_bir_lowering=False, debug=False, num_swdge_queues=4)
    yT_d = nc.declare_dram_parameter("yT", [P, M], BF16, isOutput=False)
    xT_d = nc.declare_dram_parameter("xT", [P, rows], BF16, isOutput=False)
    cr_d = nc.declare_dram_parameter("cr", [5, M], BF16, isOutput=False)
    cl_d = nc.declare_dram_parameter("cl", [5, rows], BF16, isOutput=False)
    out_d = nc.declare_dram_parameter("out", [rows, M], FP16, isOutput=True)

    # activation-table ids: one set holds Sqrt, another holds Exp+Ln+Identity
    try:
        from concourse.hw_specs import get_activation_tables

        tabs = list(get_activation_tables(nc.m.arch).values())
        SQRT_SET = next(
            i for i, s in enumerate(tabs) if AF.Sqrt in s
        )
        EXPLN_SET = next(
            i for i, s in enumerate(tabs)
            if AF.Exp in s and AF.Ln in s and AF.Identity in s
        )
    except Exception:
        SQRT_SET, EXPLN_SET = 3, 6

    # The tile scheduler reorders instructions; chain each compute engine's
    # stream (sync=False ordering hints) so the carefully balanced
    # ACT/DVE/Pool interleave survives scheduling.
    prev_inst = {}

    def chain(engine, binst):
        p = prev_inst.get(engine)
        if p is not None:
            add_dep_helper(binst.ins, p.ins, sync=False, reason=f"{engine} order")
        prev_inst[engine] = binst
        return binst

    def act(*a, **k):
        return chain("act", nc.scalar.activation(*a, **k))

    cur_table = [None]

    def ensure_table(set_id):
        if cur_table[0] == set_id:
            return
        cur_table[0] = set_id
        inst = mybir.InstLoadActFuncSet(
            name=nc.get_next_instruction_name(), ins=[], outs=[],
            act_func_set_id=set_id,
        )
        chain("act", nc.scalar.add_instruction(inst))

    with tile.TileContext(nc) as tc, ExitStack() as ctx:
        pool = lambda name, bufs, space="SBUF": ctx.enter_context(
            tc.tile_pool(name=name, bufs=bufs, space=space)
        )
        const_p = pool("const", 1)
        s_p = pool("s", 5)
        es_p = pool("es", 1)
        cr_p = pool("cr", 6)
        og_p = pool("og", 3)
        scal_p = pool("scal", 8)
        mm_ps = pool("mmps", 2, space="PSUM")   # 2 x [128, 2048] f32 = 8 banks

        # resident inputs (yT pieces loaded just-in-time during block 0)
        xT = const_p.tile([P, rows], BF16)
        nc.sync.dma_start(out=xT[:], in_=xT_d[:, :])
        cl = const_p.tile([5, rows], BF16)
        nc.sync.dma_start(out=cl[:], in_=cl_d[:, :])
        half = const_p.tile([P, CHUNK], FP16)
        chain("dve", nc.vector.memset(half[:], 0.5))
        yT = const_p.tile([P, M], BF16)

        # software pipeline: finals of block b emitted during block b+1
        pending = []  # (s_tile, lnS_tile, b, g)

        def emit_final(s_t, lnS, b, g, c):
            # write into a small staging tile (not in-place) so the s tile's
            # pool slot frees on this read, not on the out-store's completion
            og = og_p.tile([P, CHUNK], FP16)
            chain(
                "dve",
                nc.vector.tensor_scalar(
                    og[:], s_t[:, c * CHUNK : (c + 1) * CHUNK], -1.0,
                    lnS[:, 0:1], op0=ALU.mult, op1=ALU.subtract
                ),
            )
            j0 = g * GROUP + c * CHUNK
            nc.sync.dma_start(
                out=out_d[b * P : (b + 1) * P, j0 : j0 + CHUNK], in_=og[:]
            )

        # ACT table batching per block: all Sqrt chunks first (the first
        # NA_BLOCK chunks of the block), then Exp/Identity/Ln (one shared
        # table) -> 2 table loads per block. The last group's exp plus the
        # partial-sum/Ln ("tail") is deferred into the next block so ACT can
        # run the next block's sqrts while Pool finishes the last group.
        NA_BLOCK = 3

        def emit_exp(s_t, part, g):
            ensure_table(EXPLN_SET)
            es = es_p.tile([P, GROUP], FP8)
            act(es[:], s_t[:], AF.Exp, scale=-1.0, accum_out=part[:, g : g + 1])

        def make_tail(b, part, s_tiles):
            def tail():
                # last group's exp split in two so Ln lands earlier
                ensure_table(EXPLN_SET)
                s_t = s_tiles[NG - 1]
                es = es_p.tile([P, GROUP], FP8)
                h = GROUP // 2
                act(es[:, 0:h], s_t[:, 0:h], AF.Exp, scale=-1.0,
                    accum_out=part[:, NG - 1 : NG])
                act(es[:, h:], s_t[:, h:], AF.Exp, scale=-1.0,
                    accum_out=part[:, NG : NG + 1])
                # partial sum + ln on ACT itself (Identity/Ln share the Exp
                # table; on DVE this would stall its in-order queue)
                junk = scal_p.tile([P, NG + 1], F32, tag="junk")
                S = scal_p.tile([P, 1], F32, tag="S")
                act(junk[:], part[:], AF.Identity, accum_out=S[:])
                lnS = scal_p.tile([P, 1], F32, tag="lnS")
                act(lnS[:], S[:], AF.Ln)
                for g in range(NG):
                    for c in range(NCP):
                        pending.append((s_tiles[g], lnS, b, g, c))
            return tail

        # corr-row chunks are prefetched one group ahead so their (small) DMA
        # transfers dispatch before the bulky out-stores on the shared DMA FIFO
        cr_tiles = {}

        def prefetch_cr(gi):  # gi = global group index, chunk-granular tiles
            if gi >= NB * NG:
                return
            for c in range(NCP):
                j0 = (gi % NG) * GROUP + c * CHUNK
                t = cr_p.tile([5, CHUNK], BF16)
                nc.sync.dma_start(out=t[:], in_=cr_d[:, j0 : j0 + CHUNK])
                cr_tiles[(gi, c)] = t

        prev_tail = None
        prefetch_cr(0)
        for b in range(NB):
            part = scal_p.tile([P, NG + 1], F32, tag="part")
            s_tiles = []
            for g in range(NG):
                prefetch_cr(b * NG + g + 1)
                for _ in range(NCP):
                    if pending:
                        emit_final(*pending.pop(0))
                s_t = s_p.tile([P, GROUP], FP16)
                s_tiles.append(s_t)
                for c in range(NCP):
                    j0 = g * GROUP + c * CHUNK
                    if b == 0:
                        nc.sync.dma_start(
                            out=yT[:, j0 : j0 + CHUNK],
                            in_=yT_d[:, j0 : j0 + CHUNK],
                        )
                    cr_t = cr_tiles.pop((b * NG + g, c))
                    mm = mm_ps.tile([P, CHUNK], F32)
                    for q in range(CHUNK // 512):
                        nc.tensor.matmul(
                            mm[:, 512 * q : 512 * (q + 1)],
                            xT[:, b * P : (b + 1) * P],
                            yT[:, j0 + 512 * q : j0 + 512 * (q + 1)],
                            start=True,
                            stop=False,
                        )
                    for q in range(CHUNK // 512):
                        nc.tensor.matmul(
                            mm[:, 512 * q : 512 * (q + 1)],
                            cl[:, b * P : (b + 1) * P],
                            cr_t[:, 512 * q : 512 * (q + 1)],
                            start=False,
                            stop=True,
                        )
                    sl = s_t[:, c * CHUNK : (c + 1) * CHUNK]
                    if g * NCP + c < NA_BLOCK:
                        ensure_table(SQRT_SET)
                        act(sl, mm[:], AF.Sqrt)
                    else:
                        chain("dve", nc.vector.tensor_copy(sl, mm[:]))
                        chain(
                            "pool",
                            nc.gpsimd.tensor_tensor(sl, sl, half[:], op=ALU.pow),
                        )
                if g == 0:
                    if prev_tail is not None:
                        prev_tail()
                        prev_tail = None
                else:
                    emit_exp(s_tiles[g - 1], part, g - 1)
            prev_tail = make_tail(b, part, s_tiles)
        prev_tail()
        while pending:
            emit_final(*pending.pop(0))

    nc.finalize()
    return nc


_NC_CACHE = {}


def _get_nc(rows, M):
    key = (rows, M)
    if key not in _NC_CACHE:
        _NC_CACHE[key] = build_nc(rows, M)
    return _NC_CACHE[key]


def _hi_lo(v32):
    hi = v32.astype(BF)
    lo = (v32 - hi.astype(np.float32)).astype(BF)
    return hi, lo


def kernel(x: np.ndarray, y: np.ndarray, std: np.ndarray) -> np.ndarray:
    x = np.ascontiguousarray(x, dtype=np.float32)
    y = np.ascontiguousarray(y, dtype=np.float32)
    std = np.ascontiguousarray(std, dtype=np.float32)
    N, M = x.shape[0], y.shape[0]
    rows = N // N_CORES

    r2 = (1.0 / (std.astype(np.float64) ** 2)).astype(np.float32)
    yhatT = np.ascontiguousarray((y.T * r2[None, :]).astype(BF))
    bhat = ((y.astype(np.float64) ** 2).sum(axis=1) * r2.astype(np.float64)).astype(
        np.float32
    )
    r2_hi, r2_lo = _hi_lo(r2)
    b_hi, b_lo = _hi_lo(bhat)
    cr = np.ascontiguousarray(np.stack([r2_hi, r2_hi, r2_lo, b_hi, b_lo]))

    a = (x.astype(np.float64) ** 2).sum(axis=1).astype(np.float32)
    a_hi, a_lo = _hi_lo(a)
    ones = np.ones_like(a_hi)
    xT_all = np.ascontiguousarray((-2.0 * x.T).astype(BF))

    in_maps = []
    for c in range(N_CORES):
        sl = slice(c * rows, (c + 1) * rows)
        cl = np.ascontiguousarray(
            np.stack([a_hi[sl], a_lo[sl], a_hi[sl], ones[sl], ones[sl]])
        )
        in_maps.append(
            {
                "yT": yhatT,
                "xT": np.ascontiguousarray(xT_all[:, sl]),
                "cr": cr,
                "cl": cl,
            }
        )

    nc = _get_nc(rows, M)
    trace = bool(int(os.environ.get("KERNEL_TRACE", "0")))
    res = run_bass_kernel_spmd(
        nc, in_maps, core_ids=list(range(N_CORES)), trace=trace
    )
    global LAST_RESULT
    LAST_RESULT = res
    return np.concatenate(
        [res.results[c]["out"].astype(np.float32) for c in range(N_CORES)], axis=0
    )


LAST_RESULT = None


# revision 21
# speedup vs baseline: 1.0660x; 1.0660x over previous
"""Trainium2 Bass kernel for MatchingLayerL2:
   out = log_softmax(-sqrt(||x_i - y_j||^2) / std_j, axis=1)

x: [4096, 128] f32, y: [32768, 128] f32, std: [32768] f32 -> out [4096, 32768] f32.

Strategy: shard rows of x across 8 cores (512 rows each); y/std replicated.
Host prepares device inputs (layout/dtype prep only, O((N+M)D) work):
  yhatT = (y * r2[:,None]).T as bf16 [128, M]   (r2 = 1/std^2)
  xT    = (-2 x_c).T as bf16 [128, 512]
  corr rows (rank-2 term a_i*r2_j + bhat_j in hi/lo bf16 splits, K=5):
    cl = [a_hi; a_lo; a_hi; 1; 1]  [5, 512]
    cr = [r2_hi; r2_hi; r2_lo; bhat_hi; bhat_lo]  [5, M]
Device per core:
  q = xT.T @ yhatT + cl.T @ cr   (PSUM f32, = r2_j * dist2_ij)
  s = sqrt(q)  (fp16; split: 1/4 of chunks on ACT Sqrt, 3/4 via
               DVE copy PSUM->SBUF fp16 then GPSIMD tensor_tensor pow 0.5 —
               GPSIMD cannot read PSUM and sqrt/exp only exist on ACT/Pool)
  S_i = sum_j exp(-s)  (ACT Exp with accum, fp8 scratch out)
  out = -s - ln(S)     (DVE tensor_scalar in-place, fp16) -> DMA fp16
Engine balance target ~143us each for ACT (exp + 1/4 sqrt),
Pool (3/4 sqrt), DVE (copies + final); PE ~110us; DMA ~118us.
"""

import os
import sys

sys.path.insert(0, "/root/.axon_site/_ro/trn_rl_repo")

import numpy as np
import ml_dtypes
from contextlib import ExitStack

import concourse.bass as bass
from concourse import bacc
import concourse.tile as tile
from concourse.tile import add_dep_helper
from concourse import mybir
from concourse.bass_utils import run_bass_kernel_spmd

F32 = mybir.dt.float32
BF16 = mybir.dt.bfloat16
FP16 = mybir.dt.float16
FP8 = mybir.dt.float8e4
AF = mybir.ActivationFunctionType
ALU = mybir.AluOpType
AX = mybir.AxisListType

N_CORES = 8
D = 128
P = 128
CHUNK = 2048          # PSUM region columns (4 banks f32)
GROUP = 8192          # columns per exp instruction / s sub-tile
BF = ml_dtypes.bfloat16


def build_nc(rows, M):
    NB = rows // P            # 4 row blocks of 128
    NG = M // GROUP           # 4 groups per block
    NCP = GROUP // CHUNK      # 4 chunks per group

    nc = bacc.Bacc("TRN2", target_bir_lowering=False, debug=False, num_swdge_queues=4)
    yT_d = nc.declare_dram_parameter("yT", [P, M], BF16, isOutput=False)
    xT_d = nc.declare_dram_parameter("xT", [P, rows], BF16, isOutput=False)
    cr_d = nc.declare_dram_parameter("cr", [5, M], BF16, isOutput=False)
    cl_d = nc.declare_dram_parameter("cl", [5, rows], BF16, isOutput=False)
    out_d = nc.declare_dram_parameter("out", [rows, M], FP16, isOutput=True)

    # activation-table ids: one set holds Sqrt, another holds Exp+Ln+Identity
    try:
        from concourse.hw_specs import get_activation_tables

        tabs = list(get_activation_tables(nc.m.arch).values())
        SQRT_SET = next(
            i for i, s in enumerate(tabs) if AF.Sqrt in s
        )
        EXPLN_SET = next(
            i for i, s in enumerate(tabs)
            if AF.Exp in s and AF.Ln in s and AF.Identity in s
        )
    except Exception:
        SQRT_SET, EXPLN_SET = 3, 6

    # The tile scheduler reorders instructions; chain each compute engine's
    # stream (sync=False ordering hints) so the carefully balanced
    # ACT/DVE/Pool interleave survives scheduling.
    prev_inst = {}

    def chain(engine, binst):
        p = prev_inst.get(engine)
        if p is not None:
            add_dep_helper(binst.ins, p.ins, sync=False, reason=f"{engine} order")
        prev_inst[engine] = binst
        return binst

    def act(*a, **k):
        return chain("act", nc.scalar.activation(*a, **k))

    cur_table = [None]

    def ensure_table(set_id):
        if cur_table[0] == set_id:
            return
        cur_table[0] = set_id
        inst = mybir.InstLoadActFuncSet(
            name=nc.get_next_instruction_name(), ins=[], outs=[],
            act_func_set_id=set_id,
        )
        chain("act", nc.scalar.add_instruction(inst))

    with tile.TileContext(nc) as tc, ExitStack() as ctx:
        pool = lambda name, bufs, space="SBUF": ctx.enter_context(
            tc.tile_pool(name=name, bufs=bufs, space=space)
        )
        const_p = pool("const", 1)
        s_p = pool("s", 6)
        es_p = pool("es", 1)
        cr_p = pool("cr", 4)
        og_p = pool("og", 3)
        scal_p = pool("scal", 8)
        mm_ps = pool("mmps", 2, space="PSUM")   # 2 x [128, 2048] f32 = 8 banks

        # resident inputs (yT pieces loaded just-in-time during block 0)
        xT = const_p.tile([P, rows], BF16)
        nc.sync.dma_start(out=xT[:], in_=xT_d[:, :])
        cl = const_p.tile([5, rows], BF16)
        nc.sync.dma_start(out=cl[:], in_=cl_d[:, :])
        half = const_p.tile([P, CHUNK], FP16)
        chain("dve", nc.vector.memset(half[:], 0.5))
        yT = const_p.tile([P, M], BF16)

        # software pipeline: finals of block b emitted during block b+1
        pending = []  # (s_tile, lnS_tile, b, g)

        def emit_final(s_t, lnS, b, g, c):
            # write into a small staging tile (not in-place) so the s tile's
            # pool slot frees on this read, not on the out-store's completion
            og = og_p.tile([P, CHUNK], FP16)
            chain(
                "dve",
                nc.vector.tensor_scalar(
                    og[:], s_t[:, c * CHUNK : (c + 1) * CHUNK], -1.0,
                    lnS[:, 0:1], op0=ALU.mult, op1=ALU.subtract
                ),
            )
            j0 = g * GROUP + c * CHUNK
            nc.sync.dma_start(
                out=out_d[b * P : (b + 1) * P, j0 : j0 + CHUNK], in_=og[:]
            )

        # ACT table batching per block: all Sqrt chunks first (the first
        # NA_BLOCK chunks of the block), then Exp/Identity/Ln (one shared
        # table) -> 2 table loads per block. The last group's exp plus the
        # partial-sum/Ln ("tail") is deferred into the next block so ACT can
        # run the next block's sqrts while Pool finishes the last group.
        NA_BLOCK = 3

        def emit_exp(s_t, part, g):
            ensure_table(EXPLN_SET)
            es = es_p.tile([P, GROUP], FP8)
            act(es[:], s_t[:], AF.Exp, scale=-1.0, accum_out=part[:, g : g + 1])

        def make_tail(b, part, s_tiles):
            def tail():
                # last group's exp split in two so Ln lands earlier
                ensure_table(EXPLN_SET)
                s_t = s_tiles[NG - 1]
                es = es_p.tile([P, GROUP], FP8)
                h = GROUP // 2
                act(es[:, 0:h], s_t[:, 0:h], AF.Exp, scale=-1.0,
                    accum_out=part[:, NG - 1 : NG])
                act(es[:, h:], s_t[:, h:], AF.Exp, scale=-1.0,
                    accum_out=part[:, NG : NG + 1])
                # partial sum + ln on ACT itself (Identity/Ln share the Exp
                # table; on DVE this would stall its in-order queue)
                junk = scal_p.tile([P, NG + 1], F32, tag="junk")
                S = scal_p.tile([P, 1], F32, tag="S")
                act(junk[:], part[:], AF.Identity, accum_out=S[:])
                lnS = scal_p.tile([P, 1], F32, tag="lnS")
                act(lnS[:], S[:], AF.Ln)
                for g in range(NG):
                    for c in range(NCP):
                        pending.append((s_tiles[g], lnS, b, g, c))
            return tail

        # corr-row chunks are prefetched one group ahead so their (small) DMA
        # transfers dispatch before the bulky out-stores on the shared DMA FIFO
        cr_tiles = {}

        def prefetch_cr(gi):  # gi = global group index, chunk-granular tiles
            if gi >= NB * NG:
                return
            for c in range(NCP):
                j0 = (gi % NG) * GROUP + c * CHUNK
                t = cr_p.tile([5, CHUNK], BF16)
                nc.sync.dma_start(out=t[:], in_=cr_d[:, j0 : j0 + CHUNK])
                cr_tiles[(gi, c)] = t

        prev_tail = None
        prefetch_cr(0)
        for b in range(NB):
            part = scal_p.tile([P, NG + 1], F32, tag="part")
            s_tiles = []
            for g in range(NG):
                prefetch_cr(b * NG + g + 1)
                s_t = s_p.tile([P, GROUP], FP16)
                s_tiles.append(s_t)
                for c in range(NCP):
                    j0 = g * GROUP + c * CHUNK
                    if b == 0:
                        nc.sync.dma_start(
                            out=yT[:, j0 : j0 + CHUNK],
                            in_=yT_d[:, j0 : j0 + CHUNK],
                        )
                    cr_t = cr_tiles.pop((b * NG + g, c))
                    mm = mm_ps.tile([P, CHUNK], F32)
                    for q in range(CHUNK // 512):
                        nc.tensor.matmul(
                            mm[:, 512 * q : 512 * (q + 1)],
                            xT[:, b * P : (b + 1) * P],
                            yT[:, j0 + 512 * q : j0 + 512 * (q + 1)],
                            start=True,
                            stop=False,
                        )
                    for q in range(CHUNK // 512):
                        nc.tensor.matmul(
                            mm[:, 512 * q : 512 * (q + 1)],
                            cl[:, b * P : (b + 1) * P],
                            cr_t[:, 512 * q : 512 * (q + 1)],
                            start=False,
                            stop=True,
                        )
                    sl = s_t[:, c * CHUNK : (c + 1) * CHUNK]
                    if g * NCP + c < NA_BLOCK:
                        ensure_table(SQRT_SET)
                        act(sl, mm[:], AF.Sqrt)
                    else:
                        chain("dve", nc.vector.tensor_copy(sl, mm[:]))
                        chain(
                            "pool",
                            nc.gpsimd.tensor_tensor(sl, sl, half[:], op=ALU.pow),
                        )
                for _ in range(NCP):
                    if pending:
                        emit_final(*pending.pop(0))
                if g == 0:
                    if prev_tail is not None:
                        prev_tail()
                        prev_tail = None
                else:
                    emit_exp(s_tiles[g - 1], part, g - 1)
            prev_tail = make_tail(b, part, s_tiles)
        prev_tail()
        while pending:
            emit_final(*pending.pop(0))

    nc.finalize()
    return nc


_NC_CACHE = {}


def _get_nc(rows, M):
    key = (rows, M)
    if key not in _NC_CACHE:
        _NC_CACHE[key] = build_nc(rows, M)
    return _NC_CACHE[key]


def _hi_lo(v32):
    hi = v32.astype(BF)
    lo = (v32 - hi.astype(np.float32)).astype(BF)
    return hi, lo


def kernel(x: np.ndarray, y: np.ndarray, std: np.ndarray) -> np.ndarray:
    x = np.ascontiguousarray(x, dtype=np.float32)
    y = np.ascontiguousarray(y, dtype=np.float32)
    std = np.ascontiguousarray(std, dtype=np.float32)
    N, M = x.shape[0], y.shape[0]
    rows = N // N_CORES

    r2 = (1.0 / (std.astype(np.float64) ** 2)).astype(np.float32)
    yhatT = np.ascontiguousarray((y.T * r2[None, :]).astype(BF))
    bhat = ((y.astype(np.float64) ** 2).sum(axis=1) * r2.astype(np.float64)).astype(
        np.float32
    )
    r2_hi, r2_lo = _hi_lo(r2)
    b_hi, b_lo = _hi_lo(bhat)
    cr = np.ascontiguousarray(np.stack([r2_hi, r2_hi, r2_lo, b_hi, b_lo]))

    a = (x.astype(np.float64) ** 2).sum(axis=1).astype(np.float32)
    a_hi, a_lo = _hi_lo(a)
    ones = np.ones_like(a_hi)
    xT_all = np.ascontiguousarray((-2.0 * x.T).astype(BF))

    in_maps = []
    for c in range(N_CORES):
        sl = slice(c * rows, (c + 1) * rows)
        cl = np.ascontiguousarray(
            np.stack([a_hi[sl], a_lo[sl], a_hi[sl], ones[sl], ones[sl]])
        )
        in_maps.append(
            {
                "yT": yhatT,
                "xT": np.ascontiguousarray(xT_all[:, sl]),
                "cr": cr,
                "cl": cl,
            }
        )

    nc = _get_nc(rows, M)
    trace = bool(int(os.environ.get("KERNEL_TRACE", "0")))
    res = run_bass_kernel_spmd(
        nc, in_maps, core_ids=list(range(N_CORES)), trace=trace
    )
    global LAST_RESULT
    LAST_RESULT = res
    return np.concatenate(
        [res.results[c]["out"].astype(np.float32) for c in range(N_CORES)], axis=0
    )


LAST_RESULT = None


# revision 22
# speedup vs baseline: 1.2482x; 1.1709x over previous
"""Trainium2 Bass kernel for MatchingLayerL2:
   out = log_softmax(-sqrt(||x_i - y_j||^2) / std_j, axis=1)

x: [4096, 128] f32, y: [32768, 128] f32, std: [32768] f32 -> out [4096, 32768] f32.

Strategy: shard rows of x across 8 cores (512 rows each); y/std replicated.
Host prepares device inputs (layout/dtype prep only, O((N+M)D) work):
  yhatT = (y * r2[:,None]).T as bf16 [128, M]   (r2 = 1/std^2)
  xT    = (-2 x_c).T as bf16 [128, 512]
  corr rows (rank-2 term a_i*r2_j + bhat_j in hi/lo bf16 splits, K=5):
    cl = [a_hi; a_lo; a_hi; 1; 1]  [5, 512]
    cr = [r2_hi; r2_hi; r2_lo; bhat_hi; bhat_lo]  [5, M]
Device per core (512 rows = 4 row-blocks of 128):
  q = xT.T @ yhatT + cl.T @ cr   (PSUM f32, = r2_j * dist2_ij)
  s = sqrt(q)  fp16, unit = 1024 cols: first 7 units of each block on ACT
      Sqrt, the rest via DVE copy PSUM->SBUF fp16 + GPSIMD tensor_tensor
      pow 0.5 (GPSIMD cannot read PSUM; sqrt/exp exist only on ACT/Pool)
  S_i = sum_j exp(-s)  (ACT Exp + accum, fp8 scratch out; per-8192 instrs)
  out = -s - ln(S)     (DVE tensor_scalar into og staging, fp16) -> DMA
ACT runs Sqrt in one batch per block and Exp/Identity/Ln in another
(2 table loads/block); the last group's exp + Ln defer into the next
block so ACT keeps busy while Pool finishes the block's tail.
"""

import os
import sys

sys.path.insert(0, "/root/.axon_site/_ro/trn_rl_repo")

import numpy as np
import ml_dtypes
from contextlib import ExitStack

import concourse.bass as bass
from concourse import bacc
import concourse.tile as tile
from concourse.tile import add_dep_helper
from concourse import mybir
from concourse.bass_utils import run_bass_kernel_spmd

F32 = mybir.dt.float32
BF16 = mybir.dt.bfloat16
FP16 = mybir.dt.float16
FP8 = mybir.dt.float8e4
AF = mybir.ActivationFunctionType
ALU = mybir.AluOpType
AX = mybir.AxisListType

N_CORES = 8
D = 128
P = 128
UNIT = 1024           # PSUM ring unit (2 banks f32); 4-deep pipeline
GROUP = 8192          # columns per exp instruction / s sub-tile
HALFG = 4096          # final/og/out-store granularity
NA_BLOCK = 7          # leading units of each block handled by ACT Sqrt
BF = ml_dtypes.bfloat16


def build_nc(rows, M):
    NB = rows // P            # 4 row blocks of 128
    NG = M // GROUP           # 4 groups per block
    NU = GROUP // UNIT        # 8 units per group

    nc = bacc.Bacc("TRN2", target_bir_lowering=False, debug=False, num_swdge_queues=4)
    yT_d = nc.declare_dram_parameter("yT", [P, M], BF16, isOutput=False)
    xT_d = nc.declare_dram_parameter("xT", [P, rows], BF16, isOutput=False)
    cr_d = nc.declare_dram_parameter("cr", [5, M], BF16, isOutput=False)
    cl_d = nc.declare_dram_parameter("cl", [5, rows], BF16, isOutput=False)
    out_d = nc.declare_dram_parameter("out", [rows, M], FP16, isOutput=True)

    try:
        from concourse.hw_specs import get_activation_tables

        tabs = list(get_activation_tables(nc.m.arch).values())
        SQRT_SET = next(i for i, s in enumerate(tabs) if AF.Sqrt in s)
        EXPLN_SET = next(
            i for i, s in enumerate(tabs)
            if AF.Exp in s and AF.Ln in s and AF.Identity in s
        )
    except Exception:
        SQRT_SET, EXPLN_SET = 3, 6

    # The tile scheduler reorders instructions; chain each compute engine's
    # stream (sync=False ordering hints) so the balanced interleave survives.
    prev_inst = {}

    def chain(engine, binst):
        p = prev_inst.get(engine)
        if p is not None:
            add_dep_helper(binst.ins, p.ins, sync=False, reason=f"{engine} order")
        prev_inst[engine] = binst
        return binst

    def act(*a, **k):
        return chain("act", nc.scalar.activation(*a, **k))

    cur_table = [None]

    def ensure_table(set_id):
        if cur_table[0] == set_id:
            return
        cur_table[0] = set_id
        inst = mybir.InstLoadActFuncSet(
            name=nc.get_next_instruction_name(), ins=[], outs=[],
            act_func_set_id=set_id,
        )
        chain("act", nc.scalar.add_instruction(inst))

    with tile.TileContext(nc) as tc, ExitStack() as ctx:
        pool = lambda name, bufs, space="SBUF": ctx.enter_context(
            tc.tile_pool(name=name, bufs=bufs, space=space)
        )
        const_p = pool("const", 1)
        s_p = pool("s", 5)
        es_p = pool("es", 1)
        cr_p = pool("cr", 3)
        og_p = pool("og", 3)
        scal_p = pool("scal", 8)
        mm_ps = pool("mmps", 4, space="PSUM")   # 4 x [128, 1024] f32 = 8 banks

        # resident inputs (yT pieces loaded just-in-time during block 0)
        xT = const_p.tile([P, rows], BF16)
        nc.sync.dma_start(out=xT[:], in_=xT_d[:, :])
        cl = const_p.tile([5, rows], BF16)
        nc.sync.dma_start(out=cl[:], in_=cl_d[:, :])
        half = const_p.tile([P, UNIT], FP16)
        chain("dve", nc.vector.memset(half[:], 0.5))
        yT = const_p.tile([P, M], BF16)

        pending = []  # (s_tile, lnS, b, g, h) finals awaiting emission

        def emit_final(s_t, lnS, b, g, h):
            # og staging (not in-place) so the s tile's pool slot frees on
            # this read rather than on the out-store's completion
            og = og_p.tile([P, HALFG], FP16)
            chain(
                "dve",
                nc.vector.tensor_scalar(
                    og[:], s_t[:, h * HALFG : (h + 1) * HALFG], -1.0,
                    lnS[:, 0:1], op0=ALU.mult, op1=ALU.subtract
                ),
            )
            j0 = g * GROUP + h * HALFG
            nc.sync.dma_start(
                out=out_d[b * P : (b + 1) * P, j0 : j0 + HALFG], in_=og[:]
            )

        def emit_exp(s_t, part, g):
            ensure_table(EXPLN_SET)
            es = es_p.tile([P, GROUP], FP8)
            act(es[:], s_t[:], AF.Exp, scale=-1.0, accum_out=part[:, g : g + 1])

        def make_tail(b, part, s_tiles):
            def tail():
                # last group's exp split in two so Ln lands earlier
                ensure_table(EXPLN_SET)
                s_t = s_tiles[NG - 1]
                es = es_p.tile([P, GROUP], FP8)
                act(es[:, 0:HALFG], s_t[:, 0:HALFG], AF.Exp, scale=-1.0,
                    accum_out=part[:, NG - 1 : NG])
                act(es[:, HALFG:], s_t[:, HALFG:], AF.Exp, scale=-1.0,
                    accum_out=part[:, NG : NG + 1])
                # partial sum + ln on ACT itself (Identity/Ln share the Exp
                # table; on DVE this would stall its in-order queue)
                junk = scal_p.tile([P, NG + 1], F32, tag="junk")
                S = scal_p.tile([P, 1], F32, tag="S")
                act(junk[:], part[:], AF.Identity, accum_out=S[:])
                lnS = scal_p.tile([P, 1], F32, tag="lnS")
                act(lnS[:], S[:], AF.Ln)
                for g in range(NG):
                    for h in range(2):
                        pending.append((s_tiles[g], lnS, b, g, h))
            return tail

        # corr-row tiles prefetched one group ahead so their small DMAs
        # dispatch before the bulky out-stores on the shared DMA FIFO
        cr_tiles = {}

        def prefetch_cr(gi):  # gi = global group index; 2 tiles of [5, 4096]
            if gi >= NB * NG:
                return
            for hf in range(2):
                j0 = (gi % NG) * GROUP + hf * HALFG
                t = cr_p.tile([5, HALFG], BF16)
                nc.sync.dma_start(out=t[:], in_=cr_d[:, j0 : j0 + HALFG])
                cr_tiles[(gi, hf)] = t

        prev_tail = None
        prefetch_cr(0)
        for b in range(NB):
            part = scal_p.tile([P, NG + 1], F32, tag="part")
            s_tiles = []
            for g in range(NG):
                prefetch_cr(b * NG + g + 1)
                # finals first: they free s-tile slots the copies below need
                for _ in range(2):
                    if pending:
                        emit_final(*pending.pop(0))
                s_t = s_p.tile([P, GROUP], FP16)
                s_tiles.append(s_t)
                for u in range(NU):
                    j0 = g * GROUP + u * UNIT
                    if b == 0 and u % 2 == 0:
                        nc.sync.dma_start(
                            out=yT[:, j0 : j0 + 2 * UNIT],
                            in_=yT_d[:, j0 : j0 + 2 * UNIT],
                        )
                    cr_t = cr_tiles[(b * NG + g, u // (HALFG // UNIT))]
                    co = (u % (HALFG // UNIT)) * UNIT
                    mm = mm_ps.tile([P, UNIT], F32)
                    for q in range(UNIT // 512):
                        nc.tensor.matmul(
                            mm[:, 512 * q : 512 * (q + 1)],
                            xT[:, b * P : (b + 1) * P],
                            yT[:, j0 + 512 * q : j0 + 512 * (q + 1)],
                            start=True,
                            stop=False,
                        )
                    for q in range(UNIT // 512):
                        nc.tensor.matmul(
                            mm[:, 512 * q : 512 * (q + 1)],
                            cl[:, b * P : (b + 1) * P],
                            cr_t[:, co + 512 * q : co + 512 * (q + 1)],
                            start=False,
                            stop=True,
                        )
                    sl = s_t[:, u * UNIT : (u + 1) * UNIT]
                    if g * NU + u < NA_BLOCK:
                        ensure_table(SQRT_SET)
                        act(sl, mm[:], AF.Sqrt)
                    else:
                        chain("dve", nc.vector.tensor_copy(sl, mm[:]))
                        chain(
                            "pool",
                            nc.gpsimd.tensor_tensor(sl, sl, half[:], op=ALU.pow),
                        )
                if g == 0:
                    if prev_tail is not None:
                        prev_tail()
                        prev_tail = None
                else:
                    emit_exp(s_tiles[g - 1], part, g - 1)
            prev_tail = make_tail(b, part, s_tiles)
        prev_tail()
        while pending:
            emit_final(*pending.pop(0))

    nc.finalize()
    return nc


_NC_CACHE = {}


def _get_nc(rows, M):
    key = (rows, M)
    if key not in _NC_CACHE:
        _NC_CACHE[key] = build_nc(rows, M)
    return _NC_CACHE[key]


def _hi_lo(v32):
    hi = v32.astype(BF)
    lo = (v32 - hi.astype(np.float32)).astype(BF)
    return hi, lo


def kernel(x: np.ndarray, y: np.ndarray, std: np.ndarray) -> np.ndarray:
    x = np.ascontiguousarray(x, dtype=np.float32)
    y = np.ascontiguousarray(y, dtype=np.float32)
    std = np.ascontiguousarray(std, dtype=np.float32)
    N, M = x.shape[0], y.shape[0]
    rows = N // N_CORES

    r2 = (1.0 / (std.astype(np.float64) ** 2)).astype(np.float32)
    yhatT = np.ascontiguousarray((y.T * r2[None, :]).astype(BF))
    bhat = ((y.astype(np.float64) ** 2).sum(axis=1) * r2.astype(np.float64)).astype(
        np.float32
    )
    r2_hi, r2_lo = _hi_lo(r2)
    b_hi, b_lo = _hi_lo(bhat)
    cr = np.ascontiguousarray(np.stack([r2_hi, r2_hi, r2_lo, b_hi, b_lo]))

    a = (x.astype(np.float64) ** 2).sum(axis=1).astype(np.float32)
    a_hi, a_lo = _hi_lo(a)
    ones = np.ones_like(a_hi)
    xT_all = np.ascontiguousarray((-2.0 * x.T).astype(BF))

    in_maps = []
    for c in range(N_CORES):
        sl = slice(c * rows, (c + 1) * rows)
        cl = np.ascontiguousarray(
            np.stack([a_hi[sl], a_lo[sl], a_hi[sl], ones[sl], ones[sl]])
        )
        in_maps.append(
            {
                "yT": yhatT,
                "xT": np.ascontiguousarray(xT_all[:, sl]),
                "cr": cr,
                "cl": cl,
            }
        )

    nc = _get_nc(rows, M)
    trace = bool(int(os.environ.get("KERNEL_TRACE", "0")))
    res = run_bass_kernel_spmd(
        nc, in_maps, core_ids=list(range(N_CORES)), trace=trace
    )
    global LAST_RESULT
    LAST_RESULT = res
    return np.concatenate(
        [res.results[c]["out"].astype(np.float32) for c in range(N_CORES)], axis=0
    )


LAST_RESULT = None
